# revision 1
# baseline (speedup 1.0000x reference)
"""Distributed Trainium2 Bass kernel: causal multi-head attention block
(QKV proj -> causal softmax attention -> out proj -> residual -> LayerNorm)
tensor-parallel over 16 heads across 8 NeuronCores, with an AllToAll to
switch from head-sharding to sequence-sharding before the output projection.

Self-contained: callable as kernel(**inputs) with the full unsharded inputs.
"""
import numpy as np

import concourse.bacc as bacc
import concourse.mybir as mybir
import concourse.tile as tile
from concourse.bass_utils import run_bass_kernel_spmd

SEQ = 2048
D = 1024
H = 16
DK = 64
NCORES = 8
HPC = 2                 # heads per core
ROWS = SEQ // NCORES    # 256 output rows per core
QT = 512                # q-tile width
NQT = SEQ // QT         # 4
KCH = 128               # k-chunk
NKC = SEQ // KCH        # 16
NXC = D // 128          # 8 contraction chunks
EPS = 1e-5
NEG = -1e30

F32 = mybir.dt.float32
F32R = mybir.dt.float32r

ALL_CORES = [list(range(NCORES))]


def build(loop_reps=None, include_collective=True, debug_outs=False, phases=('p', 'a', 'w')):
    """Build the SPMD graph. loop_reps wraps the compute (not the collective)
    in a dynamic loop for hardware timing."""
    nc = bacc.Bacc("TRN2", target_bir_lowering=False, debug=False,
                   num_devices=NCORES)

    xt_d = nc.dram_tensor("xt", [D, SEQ], F32R, kind="ExternalInput")
    wq_d = nc.dram_tensor("wq", [D, 128], F32R, kind="ExternalInput")
    wk_d = nc.dram_tensor("wk", [D, 128], F32R, kind="ExternalInput")
    wv_d = nc.dram_tensor("wv", [D, 128], F32R, kind="ExternalInput")
    wo_d = nc.dram_tensor("wo", [D, D], F32R, kind="ExternalInput")
    bq_d = nc.dram_tensor("bq", [128, 1], F32, kind="ExternalInput")
    bk_d = nc.dram_tensor("bk", [128, 1], F32, kind="ExternalInput")
    xr_d = nc.dram_tensor("xr", [ROWS, D], F32, kind="ExternalInput")
    mask_d = nc.dram_tensor("mask", [128, 128], F32, kind="ExternalInput")
    gamma_d = nc.dram_tensor("gamma", [128, D], F32, kind="ExternalInput")
    beta_d = nc.dram_tensor("beta", [128, D], F32, kind="ExternalInput")
    out_d = nc.dram_tensor("out", [ROWS, D], F32, kind="ExternalOutput")
    if debug_outs:
        a2ain_o = nc.dram_tensor("a2ain_o", [NCORES, 128, ROWS], F32, kind="ExternalOutput")
        a2aout_o = nc.dram_tensor("a2aout_o", [NCORES, 128, ROWS], F32, kind="ExternalOutput")
        y_o = nc.dram_tensor("y_o", [2, 128, D], F32, kind="ExternalOutput")

    with tile.TileContext(nc) as tc:
        with (
            tc.tile_pool(name="sb_w", bufs=1) as sb_w,          # weights/constants
            tc.tile_pool(name="sb_act", bufs=1) as sb_act,      # persistent activations
            tc.tile_pool(name="sb_xt", bufs=2) as sb_xt,        # x^T slices
            tc.tile_pool(name="sb_e", bufs=3) as sb_e,          # exp tiles
            tc.tile_pool(name="sb_n", bufs=2) as sb_n,          # normalize tiles
            tc.tile_pool(name="sb_y", bufs=1) as sb_y,          # epilogue tiles
            tc.tile_pool(name="ps_mm", bufs=2, space="PSUM") as ps_mm,
            tc.tile_pool(name="ps_st", bufs=2, space="PSUM") as ps_st,
            tc.tile_pool(name="ps_ctx", bufs=2, space="PSUM") as ps_ctx,
            tc.tile_pool(name="dram", bufs=1, space="DRAM") as dram,
        ):
            # ---- persistent weight/constant loads (outside any timing loop)
            wq_sb = sb_w.tile([128, NXC, 128], F32R, tag="wq")
            wk_sb = sb_w.tile([128, NXC, 128], F32R, tag="wk")
            wv_sb = sb_w.tile([128, NXC, 128], F32R, tag="wv")
            wo_sb = sb_w.tile([128, NXC, D], F32R, tag="wo")
            bq_sb = sb_w.tile([128, 1], F32, tag="bq")
            bk_sb = sb_w.tile([128, 1], F32, tag="bk")
            mask_sb = sb_w.tile([128, 128], F32, tag="mask")
            xr_sb = sb_w.tile([128, 2, D], F32, tag="xr")
            gb_sb = sb_w.tile([128, D], F32, tag="gb")
            bb_sb = sb_w.tile([128, D], F32, tag="bb")
            one_col = sb_w.tile([128, 1], F32, tag="one_col")
            eps_sb = sb_w.tile([128, 1], F32, tag="eps")
            # K=64 broadcast helper: lhsT [64,64] with row0=1 rest 0;
            # rhs tiles rec64 [64, QT] with row0 = data, rest 0.
            bsel_f32 = sb_w.tile([64, 64], F32, tag="bsel_f32")
            bsel = sb_w.tile([64, 64], F32R, tag="bsel")
            zer64 = sb_w.tile([64, QT], F32, tag="zer64")
            rec64 = [sb_w.tile([64, QT], F32R, tag=f"rec64_{h}", name=f"rec64_{h}")
                     for h in range(HPC)]

            nc.sync.dma_start(out=wq_sb[:], in_=wq_d.ap().rearrange("(c p) m -> p c m", p=128))
            nc.sync.dma_start(out=wk_sb[:], in_=wk_d.ap().rearrange("(c p) m -> p c m", p=128))
            nc.sync.dma_start(out=wv_sb[:], in_=wv_d.ap().rearrange("(c p) m -> p c m", p=128))
            nc.sync.dma_start(out=wo_sb[:], in_=wo_d.ap().rearrange("(c p) m -> p c m", p=128))
            nc.sync.dma_start(out=bq_sb[:], in_=bq_d[:])
            nc.sync.dma_start(out=bk_sb[:], in_=bk_d[:])
            nc.sync.dma_start(out=mask_sb[:], in_=mask_d[:])
            nc.sync.dma_start(out=xr_sb[:], in_=xr_d.ap().rearrange("(s p) d -> p s d", p=128))
            nc.sync.dma_start(out=gb_sb[:], in_=gamma_d[:])
            nc.sync.dma_start(out=bb_sb[:], in_=beta_d[:])
            nc.vector.memset(one_col[:], 1.0)
            nc.vector.memset(eps_sb[:], EPS)
            nc.vector.memset(bsel_f32[:], 0.0)
            nc.vector.memset(bsel_f32[0:1, :], 1.0)
            nc.vector.tensor_copy(bsel[:], bsel_f32[:])
            nc.vector.memset(zer64[:], 0.0)
            for h in range(HPC):
                nc.vector.tensor_copy(rec64[h][:], zer64[:])

            a2a_in = dram.tile([NCORES, 128, ROWS], F32R, tag="a2a_in")
            a2a_out = dram.tile([NCORES, 128, ROWS], F32R, tag="a2a_out")

            xt_view = xt_d.ap().rearrange("(c p) s -> p c s", p=128)

            def body(_=None):
                do_p = "p" in phases
                do_a = "a" in phases
                # persistent per-iteration activation tiles
                qt_sb = sb_act.tile([128, SEQ], F32R, tag="qt", name="qt_sb")
                kt_sb = sb_act.tile([128, SEQ], F32R, tag="kt", name="kt_sb")
                vp = sb_act.tile([128, NKC, HPC, 65], F32R, tag="vp", name="vp")

                # ---- phase P: projections
                for st in range(NQT) if do_p else []:
                    xt_t = sb_xt.tile([128, NXC, QT], F32R, tag="xt", name=f"xt{st}")
                    nc.sync.dma_start(out=xt_t[:], in_=xt_view[:, :, st * QT:(st + 1) * QT])
                    q_ps = ps_mm.tile([128, QT], F32, tag="mm", name=f"qps{st}")
                    for c in range(NXC):
                        nc.tensor.matmul(q_ps[:], wq_sb[:, c, :], xt_t[:, c, :],
                                         start=(c == 0), stop=(c == NXC - 1))
                    nc.vector.tensor_scalar_add(qt_sb[:, st * QT:(st + 1) * QT], q_ps[:], bq_sb[:])
                    k_ps = ps_mm.tile([128, QT], F32, tag="mm", name=f"kps{st}")
                    for c in range(NXC):
                        nc.tensor.matmul(k_ps[:], wk_sb[:, c, :], xt_t[:, c, :],
                                         start=(c == 0), stop=(c == NXC - 1))
                    nc.vector.tensor_scalar_add(kt_sb[:, st * QT:(st + 1) * QT], k_ps[:], bk_sb[:])
                    for sv in range(QT // 128):
                        v_ps = ps_mm.tile([128, 128], F32, tag="mm", name=f"vps{st}_{sv}")
                        for c in range(NXC):
                            nc.tensor.matmul(v_ps[:], xt_t[:, c, sv * 128:(sv + 1) * 128],
                                             wv_sb[:, c, :], start=(c == 0),
                                             stop=(c == NXC - 1))
                        ch = st * 4 + sv
                        nc.vector.tensor_copy(
                            vp[:, ch, :, 0:64],
                            v_ps[:].rearrange("p (h d) -> p h d", h=HPC))
                        nc.vector.tensor_copy(vp[:, ch, 0, 64:65], one_col[:])
                        nc.vector.tensor_copy(vp[:, ch, 1, 64:65], one_col[:])

                # ---- phase A: attention (2 heads packed on PE rows)
                for qi in range(NQT) if do_a else []:
                    nkc_q = 4 * (qi + 1)   # causal: chunks 0..nkc_q-1
                    ctx_ps = [ps_ctx.tile([65, QT], F32, tag="ctx", name=f"ctx{qi}_{h}")
                              for h in range(HPC)]
                    # units: pairs of full chunks, then 4 single diagonal chunks
                    units = []
                    for c0 in range(0, 4 * qi, 2):
                        units.append(("pair", c0))
                    for ci in range(4 * qi, nkc_q):
                        units.append(("diag", ci))
                    for kind, ci in units:
                        if kind == "pair":
                            for h in range(HPC):
                                st2 = ps_st.tile([128, 2 * QT], F32, tag="st",
                                                 name=f"st{qi}_{ci}_{h}")
                                for half in range(2):
                                    nc.tensor.matmul(
                                        st2[:, half * QT:(half + 1) * QT],
                                        kt_sb[h * 64:(h + 1) * 64,
                                              (ci + half) * KCH:(ci + half + 1) * KCH],
                                        qt_sb[h * 64:(h + 1) * 64, qi * QT:(qi + 1) * QT],
                                        start=True, stop=True)
                                e2 = sb_e.tile([128, 2 * QT], F32R, tag="e",
                                               name=f"e{qi}_{ci}_{h}")
                                nc.scalar.activation(e2[:], st2[:],
                                                     mybir.ActivationFunctionType.Exp,
                                                     scale=1.0 / 8.0)
                                for half in range(2):
                                    nc.tensor.matmul(
                                        ctx_ps[h][:],
                                        vp[:, ci + half, h, :],
                                        e2[:, half * QT:(half + 1) * QT],
                                        start=(ci + half == 0), stop=False,
                                        skip_group_check=True)
                        else:
                            qs = ci * KCH - qi * QT   # trimmed cols
                            cols = QT - qs
                            for h in range(HPC):
                                st2 = ps_st.tile([128, 2 * QT], F32, tag="st",
                                                 name=f"st{qi}_{ci}_{h}")
                                nc.tensor.matmul(
                                    st2[:, 0:cols],
                                    kt_sb[h * 64:(h + 1) * 64, ci * KCH:(ci + 1) * KCH],
                                    qt_sb[h * 64:(h + 1) * 64,
                                          qi * QT + qs:(qi + 1) * QT],
                                    start=True, stop=True)
                                nc.vector.tensor_add(st2[:, 0:128], st2[:, 0:128], mask_sb[:])
                                e2 = sb_e.tile([128, 2 * QT], F32R, tag="e",
                                               name=f"e{qi}_{ci}_{h}")
                                nc.scalar.activation(e2[:, 0:cols], st2[:, 0:cols],
                                                     mybir.ActivationFunctionType.Exp,
                                                     scale=1.0 / 8.0)
                                nc.tensor.matmul(
                                    ctx_ps[h][:, qs:QT],
                                    vp[:, ci, h, :],
                                    e2[:, 0:cols],
                                    start=(ci == 0), stop=(ci == nkc_q - 1),
                                    skip_group_check=True)
                    # normalize + scatter into a2a_in
                    for h in range(HPC):
                        rec = sb_n.tile([1, QT], F32, tag="rec", name=f"rec{qi}_{h}")
                        nc.vector.reciprocal(rec[:], ctx_ps[h][64:65, :])
                        nc.vector.tensor_copy(rec64[h][0:1, :], rec[:])
                        bc_ps = ps_mm.tile([64, QT], F32, tag="mm", name=f"bcps{qi}_{h}")
                        nc.tensor.matmul(bc_ps[:], bsel[:], rec64[h][:],
                                         start=True, stop=True)
                        bc = sb_n.tile([64, QT], F32, tag="bc", name=f"bc{qi}_{h}")
                        nc.vector.tensor_copy(bc[:], bc_ps[:])
                        o_sb = sb_n.tile([64, QT], F32, tag="o", name=f"o{qi}_{h}")
                        nc.vector.tensor_tensor(o_sb[:], ctx_ps[h][0:64, :], bc[:],
                                                op=mybir.AluOpType.mult)
                        o_r = sb_n.tile([64, QT], F32R, tag="or", name=f"or{qi}_{h}")
                        nc.vector.tensor_copy(o_r[:], o_sb[:])
                        nc.sync.dma_start(
                            out=a2a_in[2 * qi:2 * qi + 2, h * 64:(h + 1) * 64, :]
                                .rearrange("b d q -> d b q"),
                            in_=o_r[:].rearrange("d (b q) -> d b q", b=2))
                return qt_sb  # unused

            def tail_body(_=None):
                if "w" not in phases:
                    return
                # ---- phase W: Wo matmul on own 256 rows + residual + LayerNorm
                ao = sb_act.tile([128, NCORES, ROWS], F32R, tag="ao", name="ao")
                nc.sync.dma_start(out=ao[:], in_=a2a_out[:].rearrange("j p q -> p j q"))
                for qs in range(2):
                    y_sb = sb_y.tile([128, D], F32, tag="y", name=f"y{qs}")
                    dbg_y = qs
                    for ot in range(2):
                        y_ps = ps_mm.tile([128, QT], F32, tag="mm", name=f"yps{qs}_{ot}")
                        for j in range(NCORES):
                            nc.tensor.matmul(y_ps[:], ao[:, j, qs * 128:(qs + 1) * 128],
                                             wo_sb[:, j, ot * QT:(ot + 1) * QT],
                                             start=(j == 0), stop=(j == NCORES - 1))
                        nc.vector.tensor_add(y_sb[:, ot * QT:(ot + 1) * QT], y_ps[:],
                                             xr_sb[:, qs, ot * QT:(ot + 1) * QT])
                    if debug_outs:
                        nc.sync.dma_start(out=y_o[qs], in_=y_sb[:])
                    # LayerNorm over the free dim (D)
                    musum = sb_y.tile([128, 1], F32, tag="musum", name=f"musum{qs}")
                    nc.vector.reduce_sum(musum[:], y_sb[:], axis=mybir.AxisListType.X)
                    mu = sb_y.tile([128, 1], F32, tag="mu", name=f"mu{qs}")
                    nc.scalar.mul(mu[:], musum[:], 1.0 / D)
                    t_sb = sb_y.tile([128, D], F32, tag="t", name=f"t{qs}")
                    nc.vector.tensor_scalar_sub(t_sb[:], y_sb[:], mu[:])
                    sc_sb = sb_y.tile([128, D], F32, tag="sc", name=f"sc{qs}")
                    ssq = sb_y.tile([128, 1], F32, tag="ssq", name=f"ssq{qs}")
                    nc.scalar.activation(sc_sb[:], t_sb[:],
                                         mybir.ActivationFunctionType.Square,
                                         accum_out=ssq[:])
                    lnv = sb_y.tile([128, 1], F32, tag="lnv", name=f"lnv{qs}")
                    nc.scalar.activation(lnv[:], ssq[:],
                                         mybir.ActivationFunctionType.Ln,
                                         scale=1.0 / D, bias=eps_sb[:])
                    rstd = sb_y.tile([128, 1], F32, tag="rstd", name=f"rstd{qs}")
                    nc.scalar.activation(rstd[:], lnv[:],
                                         mybir.ActivationFunctionType.Exp,
                                         scale=-0.5)
                    nc.vector.scalar_tensor_tensor(sc_sb[:], t_sb[:], rstd[:], gb_sb[:],
                                                   op0=mybir.AluOpType.mult,
                                                   op1=mybir.AluOpType.mult)
                    nc.vector.tensor_add(sc_sb[:], sc_sb[:], bb_sb[:])
                    nc.sync.dma_start(out=out_d[qs * 128:(qs + 1) * 128, :], in_=sc_sb[:])

            if loop_reps is None:
                body()
                if include_collective:
                    nc.gpsimd.collective_compute(
                        "AllToAll", mybir.AluOpType.bypass,
                        ins=[a2a_in.opt()], outs=[a2a_out.opt()],
                        replica_groups=ALL_CORES)
                if debug_outs:
                    nc.sync.dma_start(out=a2ain_o[:], in_=a2a_in[:].bitcast(F32))
                    nc.sync.dma_start(out=a2aout_o[:], in_=a2a_out[:].bitcast(F32))
                tail_body()
            else:
                def full(_i):
                    body()
                    tail_body()
                    if not (("p" in phases) or ("a" in phases) or ("w" in phases)):
                        nc.vector.memset(one_col[:], 1.0)
                with tc.For_i(0, loop_reps, 1) as i:
                    full(i)

    nc.compile()
    return nc


def make_in_maps(x, Wq, bq, Wk, bk, Wv, bv, Wo, bo, gamma, beta):
    x = np.asarray(x, np.float32)
    xt = np.ascontiguousarray(x.T)
    kk = np.arange(128, dtype=np.int64)[:, None]
    qq = np.arange(128, dtype=np.int64)[None, :]
    mask = np.where(kk <= qq, 0.0, NEG).astype(np.float32)
    Wo_c = np.ascontiguousarray(np.asarray(Wo, np.float32))
    # bv passes through softmax-weighted sums unchanged (rows sum to 1),
    # so its contribution to y is the constant row bv @ Wo; fold into bo.
    bo_eff = (np.asarray(bo, np.float32)
              + np.asarray(bv, np.float32) @ Wo_c).astype(np.float32)
    gamma_b = np.ascontiguousarray(
        np.broadcast_to(np.asarray(gamma, np.float32).reshape(1, D), (128, D)))
    beta_b = np.ascontiguousarray(
        np.broadcast_to(np.asarray(beta, np.float32).reshape(1, D), (128, D)))
    in_maps = []
    for i in range(NCORES):
        cs = slice(128 * i, 128 * (i + 1))
        rs = slice(ROWS * i, ROWS * (i + 1))
        in_maps.append({
            "xt": xt,
            "wq": np.ascontiguousarray(np.asarray(Wq, np.float32)[:, cs]),
            "wk": np.ascontiguousarray(np.asarray(Wk, np.float32)[:, cs]),
            "wv": np.ascontiguousarray(np.asarray(Wv, np.float32)[:, cs]),
            "wo": Wo_c,
            "bq": np.ascontiguousarray(np.asarray(bq, np.float32)[cs]).reshape(128, 1),
            "bk": np.ascontiguousarray(np.asarray(bk, np.float32)[cs]).reshape(128, 1),
            "xr": np.ascontiguousarray(x[rs, :] + bo_eff),
            "mask": mask,
            "gamma": gamma_b,
            "beta": beta_b,
        })
    return in_maps


_nc_cache = {}


def get_nc(loop_reps=None, include_collective=True, phases=("p", "a", "w")):
    key = (loop_reps, include_collective, tuple(phases))
    if key not in _nc_cache:
        _nc_cache[key] = build(loop_reps, include_collective, phases=phases)
    return _nc_cache[key]


def kernel(x, Wq, bq, Wk, bk, Wv, bv, Wo, bo, gamma, beta):
    nc = get_nc()
    in_maps = make_in_maps(x, Wq, bq, Wk, bk, Wv, bv, Wo, bo, gamma, beta)
    res = run_bass_kernel_spmd(nc, in_maps, core_ids=list(range(NCORES)))
    out = np.concatenate([res.results[i]["out"] for i in range(NCORES)], axis=0)
    return np.ascontiguousarray(out.astype(np.float32))

